# revision 21
# baseline (speedup 1.0000x reference)
"""PointNet MLP (3 x conv1x1+BN+ReLU, final valid-mask) on 8 TRN2 cores.

Sharding: compacted-column parallel. The valid mask keeps ~70% of the
4096*128 = 524288 point-neighbor columns; masked columns are exactly 0 in
the reference output. Host gathers the valid columns, splits them evenly
across 8 cores, device computes only those, host scatters into zeros.

Numerics: plain fp16 weights/activations with f32 PSUM accumulation and
fp16 output (end-to-end rel err ~7e-4 vs the 2e-2 gate).

Device schedule: supersteps of 2048 stream columns, software-pipelined 3
stages deep.  Steady period ~2.55us is jointly bound by cold-PE matmul
cycles (~2.54us) and the PSUM-drain work on ACT+DVE (~2.48us); the run
total is bound by drain-start + balanced ACT/DVE drain totals (~54us
each).  Startup engineering matters: each HWDGE dma_start costs its
sequencer a ~0.6-2us DIRECT2D descriptor-generation instruction, so the
sync queue carries exactly two merged input DMAs (w1+x-chunk0 aliased
into one [39, 128+4M] region; w2|w3|biases packed [128,258] f16) and the
output tiles; all bulk x goes through SWDGE on the idle Q7 cores.
"""

import numpy as np

try:
    import concourse.bass as bass
except ImportError:
    import sys

    sys.path.insert(0, "/opt/trn_rl_repo")
    import concourse.bass as bass

import concourse.bacc as bacc

import concourse.mybir as mybir
from concourse import tile
from concourse.bass_utils import run_bass_kernel_spmd

F32 = mybir.dt.float32
F16 = mybir.dt.float16

N_CORES = 8
NPOINT, KNN = 4096, 128
NCOLS = NPOINT * KNN
M = 512
ITER_COLS = 2 * M        # 1024 stream columns per iteration
SS_COLS = 2 * ITER_COLS  # 2048 stream columns per superstep
HEAD = 128               # w1 lives in x_sb cols 0:HEAD
HEAD_X = 2 * M           # x cols carried in the merged head DMA
EPS = 1e-5

_NC_CACHE = {}


def _build_nc(iters):
    assert iters % 2 == 0
    n_ss = iters // 2
    nc = bacc.Bacc("TRN2", target_bir_lowering=False)
    wx_d = nc.declare_dram_parameter("wx", [14, HEAD + HEAD_X], F16,
                                     isOutput=False)
    xp_d = nc.declare_dram_parameter("xp", [14, iters * M], F16,
                                     isOutput=False)
    wp_d = nc.declare_dram_parameter("wpack", [128, 260], F16, isOutput=False)
    out_d = nc.declare_dram_parameter("out", [128, iters * ITER_COLS], F16,
                                      isOutput=True)

    add = mybir.AluOpType.add
    vmax = mybir.AluOpType.max
    relu_fn = mybir.ActivationFunctionType.Relu

    with tile.TileContext(nc) as tc:
        with (
            tc.tile_pool(name="const", bufs=1) as cpool,
            tc.tile_pool(name="xpool", bufs=1) as xpool,
            tc.tile_pool(name="ypool", bufs=4) as ypool,
            tc.tile_pool(name="opool", bufs=4) as opool,
            tc.tile_pool(name="pspool", bufs=1, space="PSUM") as pspool,
            tc.tile_pool(name="ps3pool", bufs=2, space="PSUM") as ps3pool,
        ):
            wsb = cpool.tile([128, 260], F16, tag="wp", name="wsb")
            w2_sb = wsb[:, 0:128]
            w3_sb = wsb[:, 128:256]
            # f32 biases carried as raw bytes in 4 f16 columns
            b2_ap = wsb[:, 256:258].bitcast(F32)
            b3_ap = wsb[:, 258:260].bitcast(F32)

            xcols = HEAD + iters * M
            x_sb = xpool.tile([39, xcols], F16, tag="x", name="x_sb")
            w1a = x_sb[0:7, 0:HEAD]
            w1b = x_sb[32:39, 0:HEAD]

            # Three DIRECT2Ds on sync: [w1|x chunk0] per row group + wpack.
            nc.sync.dma_start(x_sb[0:7, 0 : HEAD + HEAD_X],
                              wx_d[0:7, :])
            nc.sync.dma_start(x_sb[32:39, 0 : HEAD + HEAD_X],
                              wx_d[7:14, :])
            nc.sync.dma_start(wsb[:, :], wp_d[:, :])
            # Bulk x via SWDGE (Q7) in uniform 2-superstep chunks: small
            # bursts so the x transfers don't starve the output DMAs on
            # the shared SDMA engines.
            lo = HEAD_X
            while lo < iters * M:
                hi = min(iters * M, lo + 4 * M)
                nc.gpsimd.dma_start(x_sb[0:7, HEAD + lo : HEAD + hi],
                                    xp_d[0:7, lo:hi])
                nc.gpsimd.dma_start(x_sb[32:39, HEAD + lo : HEAD + hi],
                                    xp_d[7:14, lo:hi])
                lo = hi

            T = {}

            def mk(s):
                T[s] = dict(
                    ps1=pspool.tile([128, 2 * M], F32, tag="ps1",
                                    name=f"ps1_{s}"),
                    ps2=pspool.tile([128, 2 * M], F32, tag="ps2",
                                    name=f"ps2_{s}"),
                    ps3a=ps3pool.tile([128, 2 * M], F32, tag="ps3",
                                      name=f"ps3a_{s}"),
                    ps3b=ps3pool.tile([128, 2 * M], F32, tag="ps3",
                                      name=f"ps3b_{s}"),
                    hi1=ypool.tile([128, 2 * M], F16, tag="hi1",
                                   name=f"hi1_{s}"),
                    hi2=ypool.tile([128, 2 * M], F16, tag="hi2",
                                   name=f"hi2_{s}"),
                    oba=opool.tile([128, 2 * M], F16, tag="ob",
                                   name=f"oba_{s}"),
                    obb=opool.tile([128, 2 * M], F16, tag="ob",
                                   name=f"obb_{s}"),
                )

            for s in range(n_ss + 2):
                if s < n_ss:
                    mk(s)
                    d = T[s]
                    c0 = HEAD + s * 2 * M
                    with tc.high_priority(offset=60):
                        nc.tensor.matmul(d["ps1"][:, 0:M], w1a,
                                         x_sb[0:7, c0 : c0 + M])
                        nc.tensor.matmul(d["ps1"][:, M : 2 * M], w1b,
                                         x_sb[32:39, c0 + M : c0 + 2 * M])
                if 1 <= s <= n_ss:
                    d = T[s - 1]
                    with tc.high_priority(offset=40):
                        nc.tensor.matmul(d["ps2"][:, 0:M], w2_sb,
                                         d["hi1"][:, 0:M])
                        nc.tensor.matmul(d["ps2"][:, M : 2 * M], w2_sb,
                                         d["hi1"][:, M : 2 * M])
                if s >= 2:
                    d = T[s - 2]
                    nc.tensor.matmul(d["ps3a"][:, 0:M], w3_sb[0:64, :],
                                     d["hi2"][0:64, 0:M])
                    nc.tensor.matmul(d["ps3a"][:, M : 2 * M], w3_sb[64:128, :],
                                     d["hi2"][64:128, 0:M])
                    nc.tensor.matmul(d["ps3b"][:, 0:M], w3_sb[0:64, :],
                                     d["hi2"][0:64, M : 2 * M])
                    nc.tensor.matmul(d["ps3b"][:, M : 2 * M], w3_sb[64:128, :],
                                     d["hi2"][64:128, M : 2 * M])

                if s < n_ss:
                    d = T[s]
                    with tc.high_priority(offset=60):
                        nc.scalar.activation(d["hi1"][:, :], d["ps1"][:, :],
                                             relu_fn)
                if 1 <= s <= n_ss:
                    d = T[s - 1]
                    with tc.high_priority(offset=40):
                        nc.scalar.activation(d["hi2"][:, :], d["ps2"][:, :],
                                             relu_fn, bias=b2_ap)
                if s >= 2:
                    d = T[s - 2]
                    if s < n_ss - 1:
                        nc.vector.tensor_scalar(d["oba"][:, :], d["ps3a"][:, :],
                                                b3_ap, 0.0, add, vmax)
                        nc.vector.tensor_scalar(d["obb"][:, :], d["ps3b"][:, :],
                                                b3_ap, 0.0, add, vmax)
                    else:
                        # endgame (last 3 cycles): ACT tapers off, so run
                        # act3a there in parallel with DVE's act3b to drain
                        # the DVE backlog faster.
                        nc.scalar.activation(d["oba"][:, :], d["ps3a"][:, :],
                                             relu_fn, bias=b3_ap)
                        nc.vector.tensor_scalar(d["obb"][:, :], d["ps3b"][:, :],
                                                b3_ap, 0.0, add, vmax)
                    o0 = (s - 2) * SS_COLS
                    nc.sync.dma_start(out_d[:, o0 : o0 + 2 * M],
                                      d["oba"][:, :])
                    nc.sync.dma_start(out_d[:, o0 + 2 * M : o0 + 4 * M],
                                      d["obb"][:, :])
                    del T[s - 2]

    nc.compile()
    return nc


def _get_nc(iters):
    if iters not in _NC_CACHE:
        _NC_CACHE[iters] = _build_nc(iters)
    return _NC_CACHE[iters]


def _fold_bn(W, b, gamma, beta, mean, var):
    inv = gamma.astype(np.float64) / np.sqrt(var.astype(np.float64) + EPS)
    Wp = (W.astype(np.float64) * inv[:, None]).astype(np.float32)
    bp = ((b.astype(np.float64) - mean.astype(np.float64)) * inv
          + beta.astype(np.float64)).astype(np.float32)
    return Wp, bp


def _prepare(inputs):
    gp = np.asarray(inputs["grouped_pc"], dtype=np.float32)
    valid = np.asarray(inputs["valid"], dtype=np.float32)

    Wp1, bp1 = _fold_bn(*(np.asarray(inputs[k], dtype=np.float32)
                          for k in ("W1", "b1", "gamma1", "beta1", "mean1", "var1")))
    Wp2, bp2 = _fold_bn(*(np.asarray(inputs[k], dtype=np.float32)
                          for k in ("W2", "b2", "gamma2", "beta2", "mean2", "var2")))
    Wp3, bp3 = _fold_bn(*(np.asarray(inputs[k], dtype=np.float32)
                          for k in ("W3", "b3", "gamma3", "beta3", "mean3", "var3")))

    lhsT1 = np.zeros((14, 128), np.float16)
    lhsT1[0:3, 0:64] = Wp1.T
    lhsT1[3:6, 64:128] = Wp1.T
    lhsT1[6, 0:64] = bp1
    lhsT1[6, 64:128] = bp1
    lhsT1[7:14] = lhsT1[0:7]

    lhsT2 = np.zeros((128, 128), np.float16)
    lhsT2[0:64, 0:64] = Wp2.T
    lhsT2[64:128, 64:128] = Wp2.T

    lhsT3 = np.zeros((128, 128), np.float16)
    lhsT3[0:64, :] = Wp3.T
    lhsT3[64:128, :] = Wp3.T

    wpack = np.zeros((128, 260), np.float16)
    wpack[:, 0:128] = lhsT2
    wpack[:, 128:256] = lhsT3
    bias_f32 = np.stack([np.concatenate([bp2, bp2]), bp3],
                        axis=1).astype(np.float32)
    wpack[:, 256:260] = bias_f32.view(np.float16)

    x = gp[0].reshape(3, NCOLS)
    vidx = np.flatnonzero(valid.reshape(NCOLS) > 0.5)
    V = len(vidx)
    Vc = -(-V // N_CORES)
    iters = max(2, 2 * (-(-Vc // SS_COLS)))
    cap = iters * ITER_COLS

    xv = x[:, vidx].astype(np.float16)

    in_maps = []
    for c in range(N_CORES):
        lo_i = c * Vc
        hi_i = min((c + 1) * Vc, V)
        n = max(0, hi_i - lo_i)
        xa = np.zeros((3, cap), np.float16)
        if n:
            xa[:, :n] = xv[:, lo_i:hi_i]
        xr = xa.reshape(3, iters, 2, M)
        xp = np.ones((14, iters, M), np.float16)
        xp[0:3] = xr[:, :, 0, :]
        xp[3:6] = xr[:, :, 1, :]
        xp[7:14] = xp[0:7]
        xp = np.ascontiguousarray(xp.reshape(14, iters * M))
        wx = np.zeros((14, HEAD + HEAD_X), np.float16)
        wx[:, 0:HEAD] = lhsT1
        wx[:, HEAD:] = xp[:, 0:HEAD_X]
        in_maps.append(
            {
                "wx": wx,
                "xp": xp,
                "wpack": wpack,
            }
        )
    return in_maps, vidx, V, Vc, iters


def _gather(results, vidx, V, Vc):
    stream = np.empty((128, V), np.float32)
    for c in range(N_CORES):
        lo_i = c * Vc
        hi_i = min((c + 1) * Vc, V)
        if hi_i <= lo_i:
            break
        stream[:, lo_i:hi_i] = results[c]["out"][:, : hi_i - lo_i]
    full = np.zeros((128, NCOLS), np.float32)
    full[:, vidx] = stream
    return full.reshape(128, NPOINT, KNN)[None]


def run_traced(trace=False, **inputs):
    in_maps, vidx, V, Vc, iters = _prepare(inputs)
    nc = _get_nc(iters)
    res = run_bass_kernel_spmd(nc, in_maps, list(range(N_CORES)), trace=trace)
    return _gather(res.results, vidx, V, Vc), res.exec_time_ns


def kernel(**inputs):
    out, _ = run_traced(trace=False, **inputs)
    return out


# revision 25
# speedup vs baseline: 1.0930x; 1.0930x over previous
"""PointNet MLP (3 x conv1x1+BN+ReLU, final valid-mask) on 8 TRN2 cores.

Sharding: compacted-column parallel. The valid mask keeps ~70% of the
4096*128 = 524288 point-neighbor columns; masked columns are exactly 0 in
the reference output. Host gathers the valid columns, splits them evenly
across 8 cores, device computes only those, host scatters into zeros.

Numerics: plain fp16 weights/activations with f32 PSUM accumulation and
fp16 output (end-to-end rel err ~7e-4 vs the 2e-2 gate).

Device schedule: supersteps of 2048 stream columns, software-pipelined 3
stages deep.  Steady period ~2.55us is jointly bound by cold-PE matmul
cycles (~2.54us) and the PSUM-drain work on ACT+DVE (~2.48us); the run
total is bound by drain-start + balanced ACT/DVE drain totals (~54us
each).  Startup engineering matters: each HWDGE dma_start costs its
sequencer a ~0.6-2us DIRECT2D descriptor-generation instruction, so the
sync queue carries exactly two merged input DMAs (w1+x-chunk0 aliased
into one [39, 128+4M] region; w2|w3|biases packed [128,258] f16) and the
output tiles; all bulk x goes through SWDGE on the idle Q7 cores.
"""

import numpy as np

try:
    import concourse.bass as bass
except ImportError:
    import sys

    sys.path.insert(0, "/opt/trn_rl_repo")
    import concourse.bass as bass

import concourse.bacc as bacc

import concourse.mybir as mybir
from concourse import tile
from concourse.bass_utils import run_bass_kernel_spmd

F32 = mybir.dt.float32
F16 = mybir.dt.float16

N_CORES = 8
NPOINT, KNN = 4096, 128
NCOLS = NPOINT * KNN
M = 512
ITER_COLS = 2 * M        # 1024 stream columns per iteration
SS_COLS = 2 * ITER_COLS  # 2048 stream columns per superstep
HEAD = 128               # w1 lives in x_sb cols 0:HEAD
HEAD_X = 4 * M           # x cols carried in the merged head DMA
EPS = 1e-5

_NC_CACHE = {}


def _build_nc(iters):
    assert iters % 2 == 0
    n_ss = iters // 2
    nc = bacc.Bacc("TRN2", target_bir_lowering=False)
    wx_d = nc.declare_dram_parameter("wx", [14, HEAD + HEAD_X // 2], F16,
                                     isOutput=False)
    xp_d = nc.declare_dram_parameter("xp", [14, iters * M // 2], F16,
                                     isOutput=False)
    wp_d = nc.declare_dram_parameter("wpack", [128, 260], F16, isOutput=False)
    out_d = nc.declare_dram_parameter("out", [128, iters * ITER_COLS], F16,
                                      isOutput=True)

    add = mybir.AluOpType.add
    vmax = mybir.AluOpType.max
    relu_fn = mybir.ActivationFunctionType.Relu

    with tile.TileContext(nc) as tc:
        with (
            tc.tile_pool(name="const", bufs=1) as cpool,
            tc.tile_pool(name="xpool", bufs=1) as xpool,
            tc.tile_pool(name="ypool", bufs=4) as ypool,
            tc.tile_pool(name="opool", bufs=4) as opool,
            tc.tile_pool(name="pspool", bufs=1, space="PSUM") as pspool,
            tc.tile_pool(name="ps3pool", bufs=2, space="PSUM") as ps3pool,
        ):
            wsb = cpool.tile([128, 260], F16, tag="wp", name="wsb")
            w2_sb = wsb[:, 0:128]
            w3_sb = wsb[:, 128:256]
            # f32 biases carried as raw bytes in 4 f16 columns
            b2_ap = wsb[:, 256:258].bitcast(F32)
            b3_ap = wsb[:, 258:260].bitcast(F32)

            xcols = HEAD + iters * M // 2
            x_sb = xpool.tile([39, xcols], F16, tag="x", name="x_sb")
            w1a = x_sb[0:7, 0:HEAD]
            w1b = x_sb[32:39, 0:HEAD]

            # Three DIRECT2Ds on sync: [w1|x chunk0] per row group + wpack.
            nc.sync.dma_start(x_sb[0:7, 0 : HEAD + HEAD_X // 2],
                              wx_d[0:7, :])
            nc.sync.dma_start(x_sb[32:39, 0 : HEAD + HEAD_X // 2],
                              wx_d[7:14, :])
            nc.sync.dma_start(wsb[:, :], wp_d[:, :])
            # Bulk x via SWDGE (Q7) in uniform 2-superstep chunks: small
            # bursts so the x transfers don't starve the output DMAs on
            # the shared SDMA engines.
            lo = HEAD_X // 2
            while lo < iters * M // 2:
                hi = min(iters * M // 2, lo + 2 * M)
                nc.gpsimd.dma_start(x_sb[0:7, HEAD + lo : HEAD + hi],
                                    xp_d[0:7, lo:hi])
                nc.gpsimd.dma_start(x_sb[32:39, HEAD + lo : HEAD + hi],
                                    xp_d[7:14, lo:hi])
                lo = hi

            T = {}

            def mk(s):
                T[s] = dict(
                    ps1=pspool.tile([128, 2 * M], F32, tag="ps1",
                                    name=f"ps1_{s}"),
                    ps2=pspool.tile([128, 2 * M], F32, tag="ps2",
                                    name=f"ps2_{s}"),
                    ps3a=ps3pool.tile([128, 2 * M], F32, tag="ps3",
                                      name=f"ps3a_{s}"),
                    ps3b=ps3pool.tile([128, 2 * M], F32, tag="ps3",
                                      name=f"ps3b_{s}"),
                    hi1=ypool.tile([128, 2 * M], F16, tag="hi1",
                                   name=f"hi1_{s}"),
                    hi2=ypool.tile([128, 2 * M], F16, tag="hi2",
                                   name=f"hi2_{s}"),
                    oba=opool.tile([128, 2 * M], F16, tag="ob",
                                   name=f"oba_{s}"),
                    obb=opool.tile([128, 2 * M], F16, tag="ob",
                                   name=f"obb_{s}"),
                )

            for s in range(n_ss + 2):
                if s < n_ss:
                    mk(s)
                    d = T[s]
                    c0 = HEAD + s * M
                    with tc.high_priority(offset=60):
                        nc.tensor.matmul(d["ps1"][:, 0:M], w1a,
                                         x_sb[0:7, c0 : c0 + M])
                        nc.tensor.matmul(d["ps1"][:, M : 2 * M], w1b,
                                         x_sb[32:39, c0 : c0 + M])
                if 1 <= s <= n_ss:
                    d = T[s - 1]
                    with tc.high_priority(offset=40):
                        nc.tensor.matmul(d["ps2"][:, 0:M], w2_sb,
                                         d["hi1"][:, 0:M])
                        nc.tensor.matmul(d["ps2"][:, M : 2 * M], w2_sb,
                                         d["hi1"][:, M : 2 * M])
                if s >= 2:
                    d = T[s - 2]
                    nc.tensor.matmul(d["ps3a"][:, 0:M], w3_sb[0:64, :],
                                     d["hi2"][0:64, 0:M])
                    nc.tensor.matmul(d["ps3a"][:, M : 2 * M], w3_sb[64:128, :],
                                     d["hi2"][64:128, 0:M])
                    nc.tensor.matmul(d["ps3b"][:, 0:M], w3_sb[0:64, :],
                                     d["hi2"][0:64, M : 2 * M])
                    nc.tensor.matmul(d["ps3b"][:, M : 2 * M], w3_sb[64:128, :],
                                     d["hi2"][64:128, M : 2 * M])

                if s < n_ss:
                    d = T[s]
                    with tc.high_priority(offset=60):
                        nc.scalar.activation(d["hi1"][:, :], d["ps1"][:, :],
                                             relu_fn)
                if 1 <= s <= n_ss:
                    d = T[s - 1]
                    with tc.high_priority(offset=40):
                        nc.scalar.activation(d["hi2"][:, :], d["ps2"][:, :],
                                             relu_fn, bias=b2_ap)
                if s >= 2:
                    d = T[s - 2]
                    if s < n_ss:
                        nc.vector.tensor_scalar(d["oba"][:, :], d["ps3a"][:, :],
                                                b3_ap, 0.0, add, vmax)
                        nc.vector.tensor_scalar(d["obb"][:, :], d["ps3b"][:, :],
                                                b3_ap, 0.0, add, vmax)
                    else:
                        # epilogue: ACT is idle here; run the two act3s in
                        # parallel instead of serially on DVE.
                        nc.scalar.activation(d["oba"][:, :], d["ps3a"][:, :],
                                             relu_fn, bias=b3_ap)
                        nc.vector.tensor_scalar(d["obb"][:, :], d["ps3b"][:, :],
                                                b3_ap, 0.0, add, vmax)
                    o0 = (s - 2) * SS_COLS
                    nc.sync.dma_start(out_d[:, o0 : o0 + 2 * M],
                                      d["oba"][:, :])
                    nc.sync.dma_start(out_d[:, o0 + 2 * M : o0 + 4 * M],
                                      d["obb"][:, :])
                    del T[s - 2]

    nc.compile()
    return nc


def _get_nc(iters):
    if iters not in _NC_CACHE:
        _NC_CACHE[iters] = _build_nc(iters)
    return _NC_CACHE[iters]


def _fold_bn(W, b, gamma, beta, mean, var):
    inv = gamma.astype(np.float64) / np.sqrt(var.astype(np.float64) + EPS)
    Wp = (W.astype(np.float64) * inv[:, None]).astype(np.float32)
    bp = ((b.astype(np.float64) - mean.astype(np.float64)) * inv
          + beta.astype(np.float64)).astype(np.float32)
    return Wp, bp


def _prepare(inputs):
    gp = np.asarray(inputs["grouped_pc"], dtype=np.float32)
    valid = np.asarray(inputs["valid"], dtype=np.float32)

    Wp1, bp1 = _fold_bn(*(np.asarray(inputs[k], dtype=np.float32)
                          for k in ("W1", "b1", "gamma1", "beta1", "mean1", "var1")))
    Wp2, bp2 = _fold_bn(*(np.asarray(inputs[k], dtype=np.float32)
                          for k in ("W2", "b2", "gamma2", "beta2", "mean2", "var2")))
    Wp3, bp3 = _fold_bn(*(np.asarray(inputs[k], dtype=np.float32)
                          for k in ("W3", "b3", "gamma3", "beta3", "mean3", "var3")))

    lhsT1 = np.zeros((14, 128), np.float16)
    lhsT1[0:3, 0:64] = Wp1.T
    lhsT1[3:6, 64:128] = Wp1.T
    lhsT1[6, 0:64] = bp1
    lhsT1[6, 64:128] = bp1
    lhsT1[7:14] = lhsT1[0:7]

    lhsT2 = np.zeros((128, 128), np.float16)
    lhsT2[0:64, 0:64] = Wp2.T
    lhsT2[64:128, 64:128] = Wp2.T

    lhsT3 = np.zeros((128, 128), np.float16)
    lhsT3[0:64, :] = Wp3.T
    lhsT3[64:128, :] = Wp3.T

    wpack = np.zeros((128, 260), np.float16)
    wpack[:, 0:128] = lhsT2
    wpack[:, 128:256] = lhsT3
    bias_f32 = np.stack([np.concatenate([bp2, bp2]), bp3],
                        axis=1).astype(np.float32)
    wpack[:, 256:260] = bias_f32.view(np.float16)

    x = gp[0].reshape(3, NCOLS)
    vidx = np.flatnonzero(valid.reshape(NCOLS) > 0.5)
    V = len(vidx)
    Vc = -(-V // N_CORES)
    iters = max(2, 2 * (-(-Vc // SS_COLS)))
    cap = iters * ITER_COLS

    xv = x[:, vidx].astype(np.float16)

    in_maps = []
    for c in range(N_CORES):
        lo_i = c * Vc
        hi_i = min((c + 1) * Vc, V)
        n = max(0, hi_i - lo_i)
        xa = np.zeros((3, cap), np.float16)
        if n:
            xa[:, :n] = xv[:, lo_i:hi_i]
        xr = xa.reshape(3, iters, 2, M)
        xp = np.ones((14, iters // 2, M), np.float16)
        xp[0:3] = xr[:, 0::2, 0, :]
        xp[3:6] = xr[:, 0::2, 1, :]
        xp[7:10] = xr[:, 1::2, 0, :]
        xp[10:13] = xr[:, 1::2, 1, :]
        xp = np.ascontiguousarray(xp.reshape(14, iters * M // 2))
        wx = np.zeros((14, HEAD + HEAD_X // 2), np.float16)
        wx[:, 0:HEAD] = lhsT1
        wx[:, HEAD:] = xp[:, 0 : HEAD_X // 2]
        in_maps.append(
            {
                "wx": wx,
                "xp": xp,
                "wpack": wpack,
            }
        )
    return in_maps, vidx, V, Vc, iters


def _gather(results, vidx, V, Vc):
    stream = np.empty((128, V), np.float32)
    for c in range(N_CORES):
        lo_i = c * Vc
        hi_i = min((c + 1) * Vc, V)
        if hi_i <= lo_i:
            break
        stream[:, lo_i:hi_i] = results[c]["out"][:, : hi_i - lo_i]
    full = np.zeros((128, NCOLS), np.float32)
    full[:, vidx] = stream
    return full.reshape(128, NPOINT, KNN)[None]


def run_traced(trace=False, **inputs):
    in_maps, vidx, V, Vc, iters = _prepare(inputs)
    nc = _get_nc(iters)
    res = run_bass_kernel_spmd(nc, in_maps, list(range(N_CORES)), trace=trace)
    return _gather(res.results, vidx, V, Vc), res.exec_time_ns


def kernel(**inputs):
    out, _ = run_traced(trace=False, **inputs)
    return out
